# revision 5
# baseline (speedup 1.0000x reference)
"""Causal GQA cross-attention kernel for Trainium2, 8-core SPMD.

Problem: q [2, 2048, 16, 128] f32, kv [2, 2048, 2, 8, 128] f32 ->
out [2, 2048, 16, 128] f32; causal mask (Sq == Sk), GQA with 2 q heads
per kv head, softmax scale 1/sqrt(128).

Sharding: 2 batches x 4 kv-head-pairs -> 8 cores. Each core gets 4 q
heads + 2 kv heads (its GQA groups), computes attention locally; no
collectives. Host splits/gathers.

Host pre-packs all device inputs so the kernel does zero on-chip
transposes or casts:
  - qT [NQH, D, SQ] bf16 (per-head transposed Q)
  - kT [NKVH, D, SK] bf16 (per-group transposed K)
  - va [NKVH, P, NKB, D+1] bf16: V rearranged to (k%128, k//128, d) with
    a baked-in ones column at d=D (yields softmax denominators for free)

Per-core algorithm (per q head):
  - S^T[k, q] = (K^T block).T @ Q^T via PE, two k-blocks into one
    [128, 1024] 2-bank PSUM tile per q superblock of 512.
  - P^T = exp(S^T * scale): split across two engines to balance load.
    Most pairs run on ACT ([128,1024] Exp pass, out bf16). A subset of
    off-diagonal pairs runs on the otherwise-idle DVE via Schraudolph's
    bit-trick exp: i32(round(x*2^23/ln2 + (127*2^23 - C))) bitcast to
    f32 approximates exp(x) to ~1.5% RMS; softmax normalization cancels
    most of that (verified end-to-end ~5e-3 rel err).
  - Causal masking for the two diagonal pairs: in-place gpsimd
    affine_select (zero-fill above the diagonal).
  - PV: out[q, d|denom] += (P^T block).T @ [V | ones] (PSUM f32
    accumulate). Causal block skipping: only k blocks <= q block.
  - Per (head, superblock): 4 PSUM->SBUF copies into one [128, 4*129]
    tile, one DMA store. Host divides by the denom column.
"""

import math
import os
import sys

import numpy as np

sys.path.insert(0, "/opt/trn_rl_repo")

import ml_dtypes  # noqa: E402

import concourse.bass as bass  # noqa: E402
import concourse.mybir as mybir  # noqa: E402
import concourse.tile as tile  # noqa: E402
from concourse import bacc  # noqa: E402
from concourse.bass_utils import run_bass_kernel_spmd  # noqa: E402

B, SQ, SK, H, HKV, D = 2, 2048, 2048, 16, 8, 128
NCORES = 8
NQH = H * B // NCORES  # 4 q heads per core
NKVH = HKV * B // NCORES  # 2 kv heads per core
P = 128
NQB = SQ // P  # 16 q blocks of 128
NSB = 4  # q superblocks of 512
SBW = 512
NKB = SK // P  # 16 k blocks
SCALE = 1.0 / math.sqrt(D)

F32 = mybir.dt.float32
BF16 = mybir.dt.bfloat16
I32 = mybir.dt.int32

BF16NP = ml_dtypes.bfloat16

# Schraudolph exp: exp(x*SCALE) ~= bitcast_f32(i32(x*SCH_A + SCH_B))
SCH_A = SCALE * (2.0**23) / math.log(2.0)
SCH_B = 127.0 * 2.0**23 - 486411.0

LAST_RESULTS = None
_CACHE = {}


def build_module():
    nc = bacc.Bacc(None, target_bir_lowering=False)

    q_d = nc.dram_tensor("q", [NQH, D, SQ], BF16, kind="ExternalInput")
    k_d = nc.dram_tensor("k", [NKVH, D, SK], BF16, kind="ExternalInput")
    v_d = nc.dram_tensor("v", [NKVH, P, NKB, D + 1], BF16, kind="ExternalInput")
    o_d = nc.dram_tensor("o", [NQH, NSB, P, 4 * (D + 1)], F32, kind="ExternalOutput")

    with tile.TileContext(nc) as tc:
        with (
            tc.tile_pool(name="kt", bufs=2) as ktp,
            tc.tile_pool(name="qt", bufs=2) as qtp,
            tc.tile_pool(name="vaug", bufs=2) as vap,
            tc.tile_pool(name="pt", bufs=6) as ptp,
            tc.tile_pool(name="ti", bufs=3) as tip,
            tc.tile_pool(name="outs", bufs=4) as outp,
            tc.tile_pool(name="pst", bufs=2, space="PSUM") as pstp,
            tc.tile_pool(name="ppv", bufs=4, space="PSUM") as ppvp,
        ):
            def mask_diag(pt, r0):
                # zero pt[k, half*512+q] where q - k - 128*(r0+half) < 0
                nc.gpsimd.affine_select(
                    out=pt.rearrange("p (h q) -> p h q", h=2),
                    in_=pt.rearrange("p (h q) -> p h q", h=2),
                    compare_op=mybir.AluOpType.is_ge,
                    fill=0.0,
                    base=-P * r0,
                    pattern=[[-P, 2], [1, SBW]],
                    channel_multiplier=-1,
                )

            def head_compute(h, qt, kt_g, vaug_g):
                for sb in range(NSB):
                    pvs = [
                        ppvp.tile([P, D + 1], F32, tag="ppv", name=f"pv_{h}_{sb}_{j}")
                        for j in range(4)
                    ]
                    for pair in range(2 * sb + 2):
                        st = pstp.tile([P, 2 * SBW], F32, tag="pst")
                        for half in (0, 1):
                            kb = 2 * pair + half
                            nc.tensor.matmul(
                                st[:, half * SBW : (half + 1) * SBW],
                                kt_g[:, kb * P : (kb + 1) * P],
                                qt[:, sb * SBW : (sb + 1) * SBW],
                                start=True,
                                stop=True,
                            )
                        pt = ptp.tile([P, 2 * SBW], BF16, tag="pt")
                        if pair < sb:
                            # off-diagonal pair on DVE (Schraudolph exp)
                            ti = tip.tile([P, 2 * SBW], I32, tag="ti")
                            nc.vector.tensor_scalar(
                                out=ti[:],
                                in0=st[:],
                                scalar1=float(SCH_A),
                                scalar2=float(SCH_B),
                                op0=mybir.AluOpType.mult,
                                op1=mybir.AluOpType.add,
                            )
                            nc.vector.tensor_copy(pt[:], ti[:].bitcast(F32))
                        else:
                            nc.scalar.activation(
                                pt[:],
                                st[:],
                                mybir.ActivationFunctionType.Exp,
                                scale=SCALE,
                            )
                        if pair == 2 * sb:
                            mask_diag(pt[:], 0)
                        elif pair == 2 * sb + 1:
                            mask_diag(pt[:], 2)
                        for half in (0, 1):
                            kb = 2 * pair + half
                            for j in range(4):
                                qb = 4 * sb + j
                                if kb > qb:
                                    continue
                                nc.tensor.matmul(
                                    pvs[j][:],
                                    pt[:, half * SBW + j * P : half * SBW + (j + 1) * P],
                                    vaug_g[:, kb, :],
                                    start=(kb == 0),
                                    stop=(kb == qb),
                                )
                    ot = outp.tile([P, 4, D + 1], F32, tag="outs")
                    for j in range(4):
                        nc.vector.tensor_copy(ot[:, j, :], pvs[j][:])
                    nc.sync.dma_start(
                        o_d[h, sb], ot[:].rearrange("p a b -> p (a b)")
                    )

            for g in range(NKVH):
                kt_g = ktp.tile([P, SK], BF16, tag="kt")
                nc.sync.dma_start(kt_g[:, 0:SBW], k_d[g, :, 0:SBW])
                nc.sync.dma_start(kt_g[:, SBW:SK], k_d[g, :, SBW:SK])
                vaug_g = vap.tile([P, NKB, D + 1], BF16, tag="vaug")
                nc.sync.dma_start(vaug_g[:, 0:4], v_d[g, :, 0:4])
                nc.sync.dma_start(vaug_g[:, 4:NKB], v_d[g, :, 4:NKB])
                for hl in range(2):
                    h = 2 * g + hl
                    qt = qtp.tile([P, SQ], BF16, tag="qt")
                    nc.sync.dma_start(qt[:, 0:SBW], q_d[h, :, 0:SBW])
                    nc.sync.dma_start(qt[:, SBW:SQ], q_d[h, :, SBW:SQ])
                    head_compute(h, qt[:], kt_g[:], vaug_g[:])

    nc.finalize()
    return nc


def _get_module():
    if "nc" not in _CACHE:
        _CACHE["nc"] = build_module()
    return _CACHE["nc"]


def kernel(q, kv):
    global LAST_RESULTS
    q = np.asarray(q, dtype=np.float32)
    kv = np.asarray(kv, dtype=np.float32)

    nc = _get_module()
    in_maps = []
    for c in range(NCORES):
        b, j = divmod(c, 4)
        # qT: [NQH, D, SQ]
        q_s = np.ascontiguousarray(
            np.transpose(q[b][:, 4 * j : 4 * j + 4, :], (1, 2, 0))
        ).astype(BF16NP)
        # kT: [NKVH, D, SK]
        k_s = np.ascontiguousarray(
            np.transpose(kv[b][:, 0, 2 * j : 2 * j + 2, :], (1, 2, 0))
        ).astype(BF16NP)
        # va: [NKVH, P, NKB, D+1] with ones at d=D
        v_full = kv[b][:, 1, 2 * j : 2 * j + 2, :]  # [SK, 2, D]
        va = np.ones((NKVH, P, NKB, D + 1), dtype=BF16NP)
        va[..., :D] = (
            np.transpose(v_full.reshape(NKB, P, NKVH, D), (2, 1, 0, 3))
        ).astype(BF16NP)
        in_maps.append({"q": q_s, "k": k_s, "v": va})

    trace = bool(int(os.environ.get("KERNEL_TRACE", "0")))
    kwargs = {}
    tdir = os.environ.get("KERNEL_TRACE_DIR")
    if tdir:
        kwargs["tmpdir"] = tdir
    res = run_bass_kernel_spmd(
        nc, in_maps, core_ids=list(range(NCORES)), trace=trace, **kwargs
    )
    LAST_RESULTS = res

    out = np.empty((B, SQ, H, D), np.float32)
    for c in range(NCORES):
        b, j = divmod(c, 4)
        o = res.results[c]["o"].reshape(NQH, NSB, P, 4, D + 1)
        o = np.transpose(o, (0, 1, 3, 2, 4)).reshape(NQH, SQ, D + 1)
        norm = o[..., :D] / o[..., D : D + 1]
        out[b, :, 4 * j : 4 * j + 4, :] = np.transpose(norm, (1, 0, 2))
    return out


# revision 14
# speedup vs baseline: 1.0198x; 1.0198x over previous
"""Causal GQA cross-attention kernel for Trainium2, 8-core SPMD.

Problem: q [2, 2048, 16, 128] f32, kv [2, 2048, 2, 8, 128] f32 ->
out [2, 2048, 16, 128] f32; causal mask (Sq == Sk), GQA with 2 q heads
per kv head, softmax scale 1/sqrt(128).

Sharding: 2 batches x 4 kv-head-pairs -> 8 cores. Each core gets 4 q
heads + 2 kv heads (its GQA groups), computes attention locally; no
collectives. Host splits/gathers.

Host pre-packs all device inputs so the kernel does zero on-chip
transposes or casts:
  - qT [NQH, D, SQ] bf16 (per-head transposed Q)
  - kT [NKVH, D, SK] bf16 (per-group transposed K)
  - va [NKVH, P, NKB, D+1] bf16: V rearranged to (k%128, k//128, d) with
    a baked-in ones column at d=D (yields softmax denominators for free)

Per-core algorithm (per q head):
  - S^T[k, q] = (K^T block).T @ Q^T via PE, two k-blocks into one
    [128, 1024] 2-bank PSUM tile per q superblock of 512.
  - P^T = exp(S^T * scale): split across two engines to balance load.
    Most pairs run on ACT ([128,1024] Exp pass, out bf16). A subset of
    off-diagonal pairs runs on the otherwise-idle DVE via Schraudolph's
    bit-trick exp: i32(round(x*2^23/ln2 + (127*2^23 - C))) bitcast to
    f32 approximates exp(x) to ~1.5% RMS; softmax normalization cancels
    most of that (verified end-to-end ~5e-3 rel err).
  - Causal masking for the two diagonal pairs: in-place gpsimd
    affine_select (zero-fill above the diagonal).
  - PV: out[q, d|denom] += (P^T block).T @ [V | ones] (PSUM f32
    accumulate). Causal block skipping: only k blocks <= q block.
  - Per (head, superblock): 4 PSUM->SBUF copies into one [128, 4*129]
    tile, one DMA store. Host divides by the denom column.
"""

import math
import os
import sys

import numpy as np

sys.path.insert(0, "/opt/trn_rl_repo")

import ml_dtypes  # noqa: E402

import concourse.bass as bass  # noqa: E402
import concourse.mybir as mybir  # noqa: E402
import concourse.tile as tile  # noqa: E402
from concourse import bacc  # noqa: E402
from concourse.bass_utils import run_bass_kernel_spmd  # noqa: E402

B, SQ, SK, H, HKV, D = 2, 2048, 2048, 16, 8, 128
NCORES = 8
NQH = H * B // NCORES  # 4 q heads per core
NKVH = HKV * B // NCORES  # 2 kv heads per core
P = 128
NQB = SQ // P  # 16 q blocks of 128
NSB = 4  # q superblocks of 512
SBW = 512
NKB = SK // P  # 16 k blocks
SCALE = 1.0 / math.sqrt(D)

F32 = mybir.dt.float32
BF16 = mybir.dt.bfloat16
I32 = mybir.dt.int32

BF16NP = ml_dtypes.bfloat16

# Schraudolph exp: exp(x*SCALE) ~= bitcast_f32(i32(x*SCH_A + SCH_B))
SCH_A = SCALE * (2.0**23) / math.log(2.0)
SCH_B = 127.0 * 2.0**23 - 486411.0

LAST_RESULTS = None
_CACHE = {}


def _build_consts():
    # c[p, 0, :] = strict-upper -10000 (lhsT for the causal bias matmul,
    # i.e. -10000 where partition p < column), c[p, 1, :] = identity.
    c = np.zeros((P, 2, P), dtype=BF16NP)
    rows = np.arange(P)[:, None]
    cols = np.arange(P)[None, :]
    tri = np.where(rows < cols, np.float32(-10000.0), np.float32(0.0))
    c[:, 0, :] = tri.astype(BF16NP)
    c[:, 1, :] = np.eye(P, dtype=np.float32).astype(BF16NP)
    return c


_CONSTS = _build_consts()


def build_module():
    nc = bacc.Bacc(None, target_bir_lowering=False)

    q_d = nc.dram_tensor("q", [NQH, D, SQ], BF16, kind="ExternalInput")
    k_d = nc.dram_tensor("k", [NKVH, D, SK], BF16, kind="ExternalInput")
    v_d = nc.dram_tensor("v", [NKVH, P, NKB, D + 1], BF16, kind="ExternalInput")
    c_d = nc.dram_tensor("c", [P, 2, P], BF16, kind="ExternalInput")
    o_d = nc.dram_tensor("o", [NQH, NSB, P, 4 * (D + 1)], F32, kind="ExternalOutput")

    # per-head DVE (Schraudolph) pair quota per superblock; diagonal pairs
    # (the last two of each sb) always run on ACT.
    DVE_QUOTA = [0, 2, 3, 3]  # of the 2*sb off-diagonal pairs

    with tile.TileContext(nc) as tc:
        with (
            tc.tile_pool(name="const", bufs=1) as constp,
            tc.tile_pool(name="kt", bufs=2) as ktp,
            tc.tile_pool(name="qt", bufs=2) as qtp,
            tc.tile_pool(name="vaug", bufs=2) as vap,
            tc.tile_pool(name="pt", bufs=6) as ptp,
            tc.tile_pool(name="ti", bufs=3) as tip,
            tc.tile_pool(name="outs", bufs=3) as outp,
            tc.tile_pool(name="pst", bufs=2, space="PSUM") as pstp,
            tc.tile_pool(name="ppv", bufs=2, space="PSUM") as ppvp,
        ):
            # Constants for the causal bias matmul: st_diag += triu.T @ Id
            # adds -10000 where q < k inside the diagonal 128x128 block.
            # Host-provided: c[0] = strict-upper -10000, c[1] = identity.
            cst = constp.tile([P, 2, P], BF16, tag="cst")
            nc.sync.dma_start(cst[:], c_d[:])
            triu = cst[:, 0, :]
            iden = cst[:, 1, :]

            def head_compute(h, qt, kt_g, vaug_g):
                for sb in range(NSB):
                    npairs = 2 * sb + 2
                    # two half-superblock PV accumulators, one bank per qb
                    pvs = [
                        ppvp.tile([P, 2, SBW], F32, tag="ppv", name=f"pv_{h}_{sb}_{hh}")
                        for hh in range(2)
                    ]
                    ot = outp.tile([P, 4, D + 1], F32, tag="outs")
                    for pair in range(npairs):
                        st = pstp.tile([P, 2 * SBW], F32, tag="pst")
                        is_diag = pair >= 2 * sb
                        for half in (0, 1):
                            kb = 2 * pair + half
                            nc.tensor.matmul(
                                st[:, half * SBW : (half + 1) * SBW],
                                kt_g[:, kb * P : (kb + 1) * P],
                                qt[:, sb * SBW : (sb + 1) * SBW],
                                start=True,
                                stop=not is_diag,
                            )
                            if is_diag:
                                # causal bias on the diagonal block kb == qb
                                j = kb - 4 * sb
                                off = half * SBW + j * P
                                nc.tensor.matmul(
                                    st[:, off : off + P],
                                    triu,
                                    iden,
                                    start=False,
                                    stop=True,
                                )
                        pt = ptp.tile([P, 2 * SBW], BF16, tag="pt")
                        if (not is_diag) and pair < DVE_QUOTA[sb]:
                            # off-diagonal pair on DVE (Schraudolph exp)
                            ti = tip.tile([P, 2 * SBW], I32, tag="ti")
                            nc.vector.tensor_scalar(
                                out=ti[:],
                                in0=st[:],
                                scalar1=float(SCH_A),
                                scalar2=float(SCH_B),
                                op0=mybir.AluOpType.mult,
                                op1=mybir.AluOpType.add,
                            )
                            nc.vector.tensor_copy(pt[:], ti[:].bitcast(F32))
                        else:
                            nc.scalar.activation(
                                pt[:],
                                st[:],
                                mybir.ActivationFunctionType.Exp,
                                scale=SCALE,
                            )
                        for half in (0, 1):
                            kb = 2 * pair + half
                            for j in range(4):
                                qb = 4 * sb + j
                                if kb > qb:
                                    continue
                                nc.tensor.matmul(
                                    pvs[j // 2][:, j % 2, 0 : D + 1],
                                    pt[:, half * SBW + j * P : half * SBW + (j + 1) * P],
                                    vaug_g[:, kb, :],
                                    start=(kb == 0),
                                    stop=(kb == qb),
                                )
                        if pair == 2 * sb:
                            # half-superblock 0 (qb = 4sb, 4sb+1) is complete
                            nc.vector.tensor_copy(
                                ot[:, 0:2, :], pvs[0][:, :, 0 : D + 1]
                            )
                    nc.vector.tensor_copy(ot[:, 2:4, :], pvs[1][:, :, 0 : D + 1])
                    nc.sync.dma_start(
                        o_d[h, sb], ot[:].rearrange("p a b -> p (a b)")
                    )

            for g in range(NKVH):
                kt_g = ktp.tile([P, SK], BF16, tag="kt")
                vaug_g = vap.tile([P, NKB, D + 1], BF16, tag="vaug")
                # load order tuned for ramp: first superblock's operands first
                nc.sync.dma_start(kt_g[:, 0:SBW], k_d[g, :, 0:SBW])
                for hl in range(2):
                    h = 2 * g + hl
                    qt = qtp.tile([P, SQ], BF16, tag="qt")
                    nc.sync.dma_start(qt[:, 0:SBW], q_d[h, :, 0:SBW])
                    if hl == 0:
                        nc.sync.dma_start(vaug_g[:, 0:4], v_d[g, :, 0:4])
                        nc.sync.dma_start(kt_g[:, SBW:SK], k_d[g, :, SBW:SK])
                        nc.sync.dma_start(vaug_g[:, 4:NKB], v_d[g, :, 4:NKB])
                    nc.sync.dma_start(qt[:, SBW:SQ], q_d[h, :, SBW:SQ])
                    head_compute(h, qt[:], kt_g[:], vaug_g[:])

    nc.finalize()
    return nc


def _get_module():
    if "nc" not in _CACHE:
        _CACHE["nc"] = build_module()
    return _CACHE["nc"]


def kernel(q, kv):
    global LAST_RESULTS
    q = np.asarray(q, dtype=np.float32)
    kv = np.asarray(kv, dtype=np.float32)

    nc = _get_module()
    in_maps = []
    for c in range(NCORES):
        b, j = divmod(c, 4)
        # qT: [NQH, D, SQ]
        q_s = np.ascontiguousarray(
            np.transpose(q[b][:, 4 * j : 4 * j + 4, :], (1, 2, 0))
        ).astype(BF16NP)
        # kT: [NKVH, D, SK]
        k_s = np.ascontiguousarray(
            np.transpose(kv[b][:, 0, 2 * j : 2 * j + 2, :], (1, 2, 0))
        ).astype(BF16NP)
        # va: [NKVH, P, NKB, D+1] with ones at d=D
        v_full = kv[b][:, 1, 2 * j : 2 * j + 2, :]  # [SK, 2, D]
        va = np.ones((NKVH, P, NKB, D + 1), dtype=BF16NP)
        va[..., :D] = (
            np.transpose(v_full.reshape(NKB, P, NKVH, D), (2, 1, 0, 3))
        ).astype(BF16NP)
        in_maps.append({"q": q_s, "k": k_s, "v": va, "c": _CONSTS})

    trace = bool(int(os.environ.get("KERNEL_TRACE", "0")))
    kwargs = {}
    tdir = os.environ.get("KERNEL_TRACE_DIR")
    if tdir:
        kwargs["tmpdir"] = tdir
    res = run_bass_kernel_spmd(
        nc, in_maps, core_ids=list(range(NCORES)), trace=trace, **kwargs
    )
    LAST_RESULTS = res

    out = np.empty((B, SQ, H, D), np.float32)
    for c in range(NCORES):
        b, j = divmod(c, 4)
        o = res.results[c]["o"].reshape(NQH, NSB, P, 4, D + 1)
        o = np.transpose(o, (0, 1, 3, 2, 4)).reshape(NQH, SQ, D + 1)
        norm = o[..., :D] / o[..., D : D + 1]
        out[b, :, 4 * j : 4 * j + 4, :] = np.transpose(norm, (1, 0, 2))
    return out


# revision 21
# speedup vs baseline: 1.0758x; 1.0549x over previous
"""Causal GQA cross-attention kernel for Trainium2, 8-core SPMD.

Problem: q [2, 2048, 16, 128] f32, kv [2, 2048, 2, 8, 128] f32 ->
out [2, 2048, 16, 128] f32; causal mask (Sq == Sk), GQA with 2 q heads
per kv head, softmax scale 1/sqrt(128).

Sharding: 2 batches x 4 kv-head-pairs -> 8 cores. Each core gets 4 q
heads + 2 kv heads (its GQA groups), computes attention locally; no
collectives. Host splits/gathers.

Host pre-packs all device inputs so the kernel does zero on-chip
transposes or casts:
  - qT [NQH, D, SQ] bf16 (per-head transposed Q)
  - kT [NKVH, D, SK] bf16 (per-group transposed K)
  - va [NKVH, P, NKB, D+1] bf16: V rearranged to (k%128, k//128, d) with
    a baked-in ones column at d=D (yields softmax denominators for free)

Per-core algorithm (per q head):
  - S^T[k, q] = (K^T block).T @ Q^T via PE, two k-blocks into one
    [128, 1024] 2-bank PSUM tile per q superblock of 512.
  - P^T = exp(S^T * scale): split across two engines to balance load.
    Most pairs run on ACT ([128,1024] Exp pass, out bf16). A subset of
    off-diagonal pairs runs on the otherwise-idle DVE via Schraudolph's
    bit-trick exp: i32(round(x*2^23/ln2 + (127*2^23 - C))) bitcast to
    f32 approximates exp(x) to ~1.5% RMS; softmax normalization cancels
    most of that (verified end-to-end ~5e-3 rel err).
  - Causal masking for the two diagonal pairs: in-place gpsimd
    affine_select (zero-fill above the diagonal).
  - PV: out[q, d|denom] += (P^T block).T @ [V | ones] (PSUM f32
    accumulate). Causal block skipping: only k blocks <= q block.
  - Per (head, superblock): 4 PSUM->SBUF copies into one [128, 4*129]
    tile, one DMA store. Host divides by the denom column.
"""

import math
import os
import sys

import numpy as np

sys.path.insert(0, "/opt/trn_rl_repo")

import ml_dtypes  # noqa: E402

import concourse.bass as bass  # noqa: E402
import concourse.mybir as mybir  # noqa: E402
import concourse.tile as tile  # noqa: E402
from concourse import bacc  # noqa: E402
from concourse.bass_utils import run_bass_kernel_spmd  # noqa: E402

B, SQ, SK, H, HKV, D = 2, 2048, 2048, 16, 8, 128
NCORES = 8
NQH = H * B // NCORES  # 4 q heads per core
NKVH = HKV * B // NCORES  # 2 kv heads per core
P = 128
NQB = SQ // P  # 16 q blocks of 128
NSB = 4  # q superblocks of 512
SBW = 512
NKB = SK // P  # 16 k blocks
SCALE = 1.0 / math.sqrt(D)

F32 = mybir.dt.float32
BF16 = mybir.dt.bfloat16
I32 = mybir.dt.int32

BF16NP = ml_dtypes.bfloat16

# Schraudolph exp at bf16 width: exp(x*SCALE) ~= bitcast_bf16(i16(x*A + B))
SCH_A = SCALE * (2.0**7) / math.log(2.0)
SCH_B = 127.0 * 2.0**7 - 7.4
I16 = mybir.dt.int16

LAST_RESULTS = None
_CACHE = {}


def _build_consts():
    # c[p, :] = causal keep-mask for a diagonal block: 1 where col >= p.
    rows = np.arange(P)[:, None]
    cols = np.arange(P)[None, :]
    return np.where(cols >= rows, np.float32(1.0), np.float32(0.0)).astype(BF16NP)


_CONSTS = _build_consts()


def build_module():
    nc = bacc.Bacc(None, target_bir_lowering=False)

    q_d = nc.dram_tensor("q", [NQH, D, SQ], BF16, kind="ExternalInput")
    k_d = nc.dram_tensor("k", [NKVH, D, SK], BF16, kind="ExternalInput")
    v_d = nc.dram_tensor("v", [NKVH, P, NKB, D + 1], BF16, kind="ExternalInput")
    c_d = nc.dram_tensor("c", [P, P], BF16, kind="ExternalInput")
    o_d = nc.dram_tensor("o", [NQH, NSB, P, 4 * (D + 1)], F32, kind="ExternalOutput")

    # per-head DVE (Schraudolph) pair quota per superblock; diagonal pairs
    # (the last two of each sb) always run on ACT.
    DVE_QUOTA = [0, 2, 2, 3]  # of the 2*sb off-diagonal pairs

    with tile.TileContext(nc) as tc:
        with (
            tc.tile_pool(name="const", bufs=1) as constp,
            tc.tile_pool(name="kt", bufs=2) as ktp,
            tc.tile_pool(name="qt", bufs=2) as qtp,
            tc.tile_pool(name="vaug", bufs=2) as vap,
            tc.tile_pool(name="pt", bufs=6) as ptp,
            tc.tile_pool(name="ti", bufs=4) as tip,
            tc.tile_pool(name="outs", bufs=3) as outp,
            tc.tile_pool(name="pst", bufs=2, space="PSUM") as pstp,
            tc.tile_pool(name="ppv", bufs=2, space="PSUM") as ppvp,
        ):
            # Host-provided causal keep-mask for diagonal 128x128 blocks.
            dmask = constp.tile([P, P], BF16, tag="dmask")
            nc.sync.dma_start(dmask[:], c_d[:])

            def head_compute(h, qt, kt_g, vaug_g):
                for sb in range(NSB):
                    npairs = 2 * sb + 2
                    # two half-superblock PV accumulators, one bank per qb
                    pvs = [
                        ppvp.tile([P, 2, SBW], F32, tag="ppv", name=f"pv_{h}_{sb}_{hh}")
                        for hh in range(2)
                    ]
                    ot = outp.tile([P, 4, D + 1], F32, tag="outs")
                    for pair in range(npairs):
                        st = pstp.tile([P, 2 * SBW], F32, tag="pst")
                        is_diag = pair >= 2 * sb
                        for half in (0, 1):
                            kb = 2 * pair + half
                            nc.tensor.matmul(
                                st[:, half * SBW : (half + 1) * SBW],
                                kt_g[:, kb * P : (kb + 1) * P],
                                qt[:, sb * SBW : (sb + 1) * SBW],
                                start=True,
                                stop=True,
                            )
                        if (not is_diag) and pair < DVE_QUOTA[sb]:
                            # off-diagonal pair on DVE: bf16-width Schraudolph
                            # exp; the int16 result IS the bf16 bit pattern.
                            ti = tip.tile([P, 2 * SBW], I16, tag="ti")
                            nc.vector.tensor_scalar(
                                out=ti[:],
                                in0=st[:],
                                scalar1=float(SCH_A),
                                scalar2=float(SCH_B),
                                op0=mybir.AluOpType.mult,
                                op1=mybir.AluOpType.add,
                            )
                            pt = ti[:].bitcast(BF16)
                        else:
                            ptt = ptp.tile([P, 2 * SBW], BF16, tag="pt")
                            nc.scalar.activation(
                                ptt[:],
                                st[:],
                                mybir.ActivationFunctionType.Exp,
                                scale=SCALE,
                            )
                            pt = ptt[:]
                            if is_diag:
                                # mask the two diagonal 128x128 blocks
                                for half in (0, 1):
                                    kb = 2 * pair + half
                                    j = kb - 4 * sb
                                    off = half * SBW + j * P
                                    nc.vector.tensor_tensor(
                                        out=ptt[:, off : off + P],
                                        in0=ptt[:, off : off + P],
                                        in1=dmask[:],
                                        op=mybir.AluOpType.mult,
                                    )
                        for half in (0, 1):
                            kb = 2 * pair + half
                            for j in range(4):
                                qb = 4 * sb + j
                                if kb > qb:
                                    continue
                                nc.tensor.matmul(
                                    pvs[j // 2][:, j % 2, 0 : D + 1],
                                    pt[:, half * SBW + j * P : half * SBW + (j + 1) * P],
                                    vaug_g[:, kb, :],
                                    start=(kb == 0),
                                    stop=(kb == qb),
                                )
                        if pair == 2 * sb:
                            # half-superblock 0 (qb = 4sb, 4sb+1) is complete
                            nc.vector.tensor_copy(
                                ot[:, 0:2, :], pvs[0][:, :, 0 : D + 1]
                            )
                    nc.vector.tensor_copy(ot[:, 2:4, :], pvs[1][:, :, 0 : D + 1])
                    nc.sync.dma_start(
                        o_d[h, sb], ot[:].rearrange("p a b -> p (a b)")
                    )

            for g in range(NKVH):
                kt_g = ktp.tile([P, SK], BF16, tag="kt")
                vaug_g = vap.tile([P, NKB, D + 1], BF16, tag="vaug")
                # load order tuned for ramp: first superblock's operands first
                nc.sync.dma_start(kt_g[:, 0:SBW], k_d[g, :, 0:SBW])
                for hl in range(2):
                    h = 2 * g + hl
                    qt = qtp.tile([P, SQ], BF16, tag="qt")
                    nc.sync.dma_start(qt[:, 0:SBW], q_d[h, :, 0:SBW])
                    if hl == 0:
                        nc.sync.dma_start(vaug_g[:, 0:4], v_d[g, :, 0:4])
                        nc.sync.dma_start(kt_g[:, SBW:SK], k_d[g, :, SBW:SK])
                        nc.sync.dma_start(vaug_g[:, 4:NKB], v_d[g, :, 4:NKB])
                    nc.sync.dma_start(qt[:, SBW:SQ], q_d[h, :, SBW:SQ])
                    head_compute(h, qt[:], kt_g[:], vaug_g[:])

    nc.finalize()
    return nc


def _get_module():
    if "nc" not in _CACHE:
        _CACHE["nc"] = build_module()
    return _CACHE["nc"]


def kernel(q, kv):
    global LAST_RESULTS
    q = np.asarray(q, dtype=np.float32)
    kv = np.asarray(kv, dtype=np.float32)

    nc = _get_module()
    in_maps = []
    for c in range(NCORES):
        b, j = divmod(c, 4)
        # qT: [NQH, D, SQ]
        q_s = np.ascontiguousarray(
            np.transpose(q[b][:, 4 * j : 4 * j + 4, :], (1, 2, 0))
        ).astype(BF16NP)
        # kT: [NKVH, D, SK]
        k_s = np.ascontiguousarray(
            np.transpose(kv[b][:, 0, 2 * j : 2 * j + 2, :], (1, 2, 0))
        ).astype(BF16NP)
        # va: [NKVH, P, NKB, D+1] with ones at d=D
        v_full = kv[b][:, 1, 2 * j : 2 * j + 2, :]  # [SK, 2, D]
        va = np.ones((NKVH, P, NKB, D + 1), dtype=BF16NP)
        va[..., :D] = (
            np.transpose(v_full.reshape(NKB, P, NKVH, D), (2, 1, 0, 3))
        ).astype(BF16NP)
        in_maps.append({"q": q_s, "k": k_s, "v": va, "c": _CONSTS})

    trace = bool(int(os.environ.get("KERNEL_TRACE", "0")))
    kwargs = {}
    tdir = os.environ.get("KERNEL_TRACE_DIR")
    if tdir:
        kwargs["tmpdir"] = tdir
    res = run_bass_kernel_spmd(
        nc, in_maps, core_ids=list(range(NCORES)), trace=trace, **kwargs
    )
    LAST_RESULTS = res

    out = np.empty((B, SQ, H, D), np.float32)
    for c in range(NCORES):
        b, j = divmod(c, 4)
        o = res.results[c]["o"].reshape(NQH, NSB, P, 4, D + 1)
        o = np.transpose(o, (0, 1, 3, 2, 4)).reshape(NQH, SQ, D + 1)
        norm = o[..., :D] / o[..., D : D + 1]
        out[b, :, 4 * j : 4 * j + 4, :] = np.transpose(norm, (1, 0, 2))
    return out


# revision 23
# speedup vs baseline: 1.2857x; 1.1952x over previous
"""Causal GQA cross-attention kernel for Trainium2, 8-core SPMD.

Problem: q [2, 2048, 16, 128] f32, kv [2, 2048, 2, 8, 128] f32 ->
out [2, 2048, 16, 128] f32; causal mask (Sq == Sk), GQA with 2 q heads
per kv head, softmax scale 1/sqrt(128).

Sharding: 2 batches x 4 kv-head-pairs -> 8 cores. Each core gets 4 q
heads + 2 kv heads (its GQA groups), computes attention locally; no
collectives. Host splits/gathers.

Host pre-packs all device inputs so the kernel does zero on-chip
transposes or casts:
  - qT [NQH, D, SQ] bf16 (per-head transposed Q)
  - kT [NKVH, D, SK] bf16 (per-group transposed K)
  - va [NKVH, P, NKB, D+1] bf16: V rearranged to (k%128, k//128, d) with
    a baked-in ones column at d=D (yields softmax denominators for free)

Per-core algorithm (per q head):
  - S^T[k, q] = (K^T block).T @ Q^T via PE, two k-blocks into one
    [128, 1024] 2-bank PSUM tile per q superblock of 512.
  - P^T = exp(S^T * scale): split across two engines to balance load.
    Most pairs run on ACT ([128,1024] Exp pass, out bf16). A subset of
    off-diagonal pairs runs on the otherwise-idle DVE via Schraudolph's
    bit-trick exp: i32(round(x*2^23/ln2 + (127*2^23 - C))) bitcast to
    f32 approximates exp(x) to ~1.5% RMS; softmax normalization cancels
    most of that (verified end-to-end ~5e-3 rel err).
  - Causal masking for the two diagonal pairs: in-place gpsimd
    affine_select (zero-fill above the diagonal).
  - PV: out[q, d|denom] += (P^T block).T @ [V | ones] (PSUM f32
    accumulate). Causal block skipping: only k blocks <= q block.
  - Per (head, superblock): 4 PSUM->SBUF copies into one [128, 4*129]
    tile, one DMA store. Host divides by the denom column.
"""

import math
import os
import sys

import numpy as np

sys.path.insert(0, "/opt/trn_rl_repo")

import ml_dtypes  # noqa: E402

import concourse.bass as bass  # noqa: E402
import concourse.mybir as mybir  # noqa: E402
import concourse.tile as tile  # noqa: E402
from concourse import bacc  # noqa: E402
from concourse.bass_utils import run_bass_kernel_spmd  # noqa: E402

B, SQ, SK, H, HKV, D = 2, 2048, 2048, 16, 8, 128
NCORES = 8
NQH = H * B // NCORES  # 4 q heads per core
NKVH = HKV * B // NCORES  # 2 kv heads per core
P = 128
NQB = SQ // P  # 16 q blocks of 128
NSB = 4  # q superblocks of 512
SBW = 512
NKB = SK // P  # 16 k blocks
SCALE = 1.0 / math.sqrt(D)

F32 = mybir.dt.float32
BF16 = mybir.dt.bfloat16
I32 = mybir.dt.int32

BF16NP = ml_dtypes.bfloat16

# Schraudolph exp at bf16 width: exp(x*SCALE) ~= bitcast_bf16(i16(x*A + B))
SCH_A = SCALE * (2.0**7) / math.log(2.0)
SCH_B = 127.0 * 2.0**7 - 7.4
I16 = mybir.dt.int16

LAST_RESULTS = None
_CACHE = {}


def _build_consts():
    # c[p, :] = causal keep-mask for a diagonal block: 1 where col >= p.
    rows = np.arange(P)[:, None]
    cols = np.arange(P)[None, :]
    return np.where(cols >= rows, np.float32(1.0), np.float32(0.0)).astype(BF16NP)


_CONSTS = _build_consts()


def build_module():
    nc = bacc.Bacc(None, target_bir_lowering=False)

    q_d = nc.dram_tensor("q", [NQH, D, SQ], BF16, kind="ExternalInput")
    k_d = nc.dram_tensor("k", [NKVH, D, SK], BF16, kind="ExternalInput")
    v_d = nc.dram_tensor("v", [NKVH, P, NKB, D + 1], BF16, kind="ExternalInput")
    c_d = nc.dram_tensor("c", [P, P], BF16, kind="ExternalInput")
    o_d = nc.dram_tensor("o", [NQH, NSB, P, 4 * (D + 1)], F32, kind="ExternalOutput")

    # per-head DVE (Schraudolph) pair quota per superblock; diagonal pairs
    # (the last two of each sb) always run on ACT.
    DVE_QUOTA = [0, 2, 2, 3]  # of the 2*sb off-diagonal pairs

    with tile.TileContext(nc) as tc:
        with (
            tc.tile_pool(name="const", bufs=1) as constp,
            tc.tile_pool(name="kt", bufs=2) as ktp,
            tc.tile_pool(name="qt", bufs=2) as qtp,
            tc.tile_pool(name="vaug", bufs=2) as vap,
            tc.tile_pool(name="pt", bufs=8) as ptp,
            tc.tile_pool(name="ti", bufs=6) as tip,
            tc.tile_pool(name="outs", bufs=3) as outp,
            tc.tile_pool(name="pst", bufs=4, space="PSUM") as pstp,
            tc.tile_pool(name="ppv", bufs=2, space="PSUM") as ppvp,
        ):
            # Host-provided causal keep-mask for diagonal 128x128 blocks.
            dmask = constp.tile([P, P], BF16, tag="dmask")
            nc.sync.dma_start(dmask[:], c_d[:])

            def head_compute(h, qt, kt_g, vaug_g):
                for sb in range(NSB):
                    npairs = 2 * sb + 2
                    # two half-superblock PV accumulators, one bank per qb
                    pvs = [
                        ppvp.tile([P, 2, SBW], F32, tag="ppv", name=f"pv_{h}_{sb}_{hh}")
                        for hh in range(2)
                    ]
                    ot = outp.tile([P, 4, D + 1], F32, tag="outs")
                    for pair in range(npairs):
                        is_diag = pair >= 2 * sb
                        pth = [None, None]
                        for half in (0, 1):
                            kb = 2 * pair + half
                            st = pstp.tile([P, SBW], F32, tag="pst")
                            nc.tensor.matmul(
                                st[:],
                                kt_g[:, kb * P : (kb + 1) * P],
                                qt[:, sb * SBW : (sb + 1) * SBW],
                                start=True,
                                stop=True,
                            )
                            if is_diag or half == 0:
                                # ACT exp for this k-block half
                                ptt = ptp.tile([P, SBW], BF16, tag="pt")
                                nc.scalar.activation(
                                    ptt[:],
                                    st[:],
                                    mybir.ActivationFunctionType.Exp,
                                    scale=SCALE,
                                )
                                pth[half] = ptt[:]
                                if is_diag:
                                    # mask the diagonal 128x128 block kb == qb
                                    j = kb - 4 * sb
                                    nc.vector.tensor_tensor(
                                        out=ptt[:, j * P : (j + 1) * P],
                                        in0=ptt[:, j * P : (j + 1) * P],
                                        in1=dmask[:],
                                        op=mybir.AluOpType.mult,
                                    )
                            else:
                                # DVE bf16-width Schraudolph exp: the int16
                                # result IS the bf16 bit pattern.
                                ti = tip.tile([P, SBW], I16, tag="ti")
                                nc.vector.tensor_scalar(
                                    out=ti[:],
                                    in0=st[:],
                                    scalar1=float(SCH_A),
                                    scalar2=float(SCH_B),
                                    op0=mybir.AluOpType.mult,
                                    op1=mybir.AluOpType.add,
                                )
                                pth[half] = ti[:].bitcast(BF16)
                        for half in (0, 1):
                            kb = 2 * pair + half
                            for j in range(4):
                                qb = 4 * sb + j
                                if kb > qb:
                                    continue
                                nc.tensor.matmul(
                                    pvs[j // 2][:, j % 2, 0 : D + 1],
                                    pth[half][:, j * P : (j + 1) * P],
                                    vaug_g[:, kb, :],
                                    start=(kb == 0),
                                    stop=(kb == qb),
                                )
                        if pair == 2 * sb:
                            # half-superblock 0 (qb = 4sb, 4sb+1) is complete
                            nc.vector.tensor_copy(
                                ot[:, 0:2, :], pvs[0][:, :, 0 : D + 1]
                            )
                    nc.vector.tensor_copy(ot[:, 2:4, :], pvs[1][:, :, 0 : D + 1])
                    nc.sync.dma_start(
                        o_d[h, sb], ot[:].rearrange("p a b -> p (a b)")
                    )

            for g in range(NKVH):
                kt_g = ktp.tile([P, SK], BF16, tag="kt")
                vaug_g = vap.tile([P, NKB, D + 1], BF16, tag="vaug")
                # load order tuned for ramp: first superblock's operands first
                nc.sync.dma_start(kt_g[:, 0:SBW], k_d[g, :, 0:SBW])
                for hl in range(2):
                    h = 2 * g + hl
                    qt = qtp.tile([P, SQ], BF16, tag="qt")
                    nc.sync.dma_start(qt[:, 0:SBW], q_d[h, :, 0:SBW])
                    if hl == 0:
                        nc.sync.dma_start(vaug_g[:, 0:4], v_d[g, :, 0:4])
                        nc.sync.dma_start(kt_g[:, SBW:SK], k_d[g, :, SBW:SK])
                        nc.sync.dma_start(vaug_g[:, 4:NKB], v_d[g, :, 4:NKB])
                    nc.sync.dma_start(qt[:, SBW:SQ], q_d[h, :, SBW:SQ])
                    head_compute(h, qt[:], kt_g[:], vaug_g[:])

    nc.finalize()
    return nc


def _get_module():
    if "nc" not in _CACHE:
        _CACHE["nc"] = build_module()
    return _CACHE["nc"]


def kernel(q, kv):
    global LAST_RESULTS
    q = np.asarray(q, dtype=np.float32)
    kv = np.asarray(kv, dtype=np.float32)

    nc = _get_module()
    in_maps = []
    for c in range(NCORES):
        b, j = divmod(c, 4)
        # qT: [NQH, D, SQ]
        q_s = np.ascontiguousarray(
            np.transpose(q[b][:, 4 * j : 4 * j + 4, :], (1, 2, 0))
        ).astype(BF16NP)
        # kT: [NKVH, D, SK]
        k_s = np.ascontiguousarray(
            np.transpose(kv[b][:, 0, 2 * j : 2 * j + 2, :], (1, 2, 0))
        ).astype(BF16NP)
        # va: [NKVH, P, NKB, D+1] with ones at d=D
        v_full = kv[b][:, 1, 2 * j : 2 * j + 2, :]  # [SK, 2, D]
        va = np.ones((NKVH, P, NKB, D + 1), dtype=BF16NP)
        va[..., :D] = (
            np.transpose(v_full.reshape(NKB, P, NKVH, D), (2, 1, 0, 3))
        ).astype(BF16NP)
        in_maps.append({"q": q_s, "k": k_s, "v": va, "c": _CONSTS})

    trace = bool(int(os.environ.get("KERNEL_TRACE", "0")))
    kwargs = {}
    tdir = os.environ.get("KERNEL_TRACE_DIR")
    if tdir:
        kwargs["tmpdir"] = tdir
    res = run_bass_kernel_spmd(
        nc, in_maps, core_ids=list(range(NCORES)), trace=trace, **kwargs
    )
    LAST_RESULTS = res

    out = np.empty((B, SQ, H, D), np.float32)
    for c in range(NCORES):
        b, j = divmod(c, 4)
        o = res.results[c]["o"].reshape(NQH, NSB, P, 4, D + 1)
        o = np.transpose(o, (0, 1, 3, 2, 4)).reshape(NQH, SQ, D + 1)
        norm = o[..., :D] / o[..., D : D + 1]
        out[b, :, 4 * j : 4 * j + 4, :] = np.transpose(norm, (1, 0, 2))
    return out


# revision 26
# speedup vs baseline: 1.3556x; 1.0543x over previous
"""Causal GQA cross-attention kernel for Trainium2, 8-core SPMD.

Problem: q [2, 2048, 16, 128] f32, kv [2, 2048, 2, 8, 128] f32 ->
out [2, 2048, 16, 128] f32; causal mask (Sq == Sk), GQA with 2 q heads
per kv head, softmax scale 1/sqrt(128).

Sharding: 2 batches x 4 kv-head-pairs -> 8 cores. Each core gets 4 q
heads + 2 kv heads (its GQA groups), computes attention locally; no
collectives. Host splits/gathers.

Host pre-packs all device inputs so the kernel does zero on-chip
transposes or casts:
  - qT [NQH, D, SQ] bf16 (per-head transposed Q)
  - kT [NKVH, D, SK] bf16 (per-group transposed K)
  - va [NKVH, P, NKB, D+1] bf16: V rearranged to (k%128, k//128, d) with
    a baked-in ones column at d=D (yields softmax denominators for free)

Per-core algorithm (per q head):
  - S^T[k, q] = (K^T block).T @ Q^T via PE, two k-blocks into one
    [128, 1024] 2-bank PSUM tile per q superblock of 512.
  - P^T = exp(S^T * scale): split across two engines to balance load.
    Most pairs run on ACT ([128,1024] Exp pass, out bf16). A subset of
    off-diagonal pairs runs on the otherwise-idle DVE via Schraudolph's
    bit-trick exp: i32(round(x*2^23/ln2 + (127*2^23 - C))) bitcast to
    f32 approximates exp(x) to ~1.5% RMS; softmax normalization cancels
    most of that (verified end-to-end ~5e-3 rel err).
  - Causal masking for the two diagonal pairs: in-place gpsimd
    affine_select (zero-fill above the diagonal).
  - PV: out[q, d|denom] += (P^T block).T @ [V | ones] (PSUM f32
    accumulate). Causal block skipping: only k blocks <= q block.
  - Per (head, superblock): 4 PSUM->SBUF copies into one [128, 4*129]
    tile, one DMA store. Host divides by the denom column.
"""

import math
import os
import sys

import numpy as np

sys.path.insert(0, "/opt/trn_rl_repo")

import ml_dtypes  # noqa: E402

import concourse.bass as bass  # noqa: E402
import concourse.mybir as mybir  # noqa: E402
import concourse.tile as tile  # noqa: E402
from concourse import bacc  # noqa: E402
from concourse.bass_utils import run_bass_kernel_spmd  # noqa: E402

B, SQ, SK, H, HKV, D = 2, 2048, 2048, 16, 8, 128
NCORES = 8
NQH = H * B // NCORES  # 4 q heads per core
NKVH = HKV * B // NCORES  # 2 kv heads per core
P = 128
NQB = SQ // P  # 16 q blocks of 128
NSB = 4  # q superblocks of 512
SBW = 512
NKB = SK // P  # 16 k blocks
SCALE = 1.0 / math.sqrt(D)

F32 = mybir.dt.float32
BF16 = mybir.dt.bfloat16
I32 = mybir.dt.int32

BF16NP = ml_dtypes.bfloat16

# Schraudolph exp at bf16 width: exp(x*SCALE) ~= bitcast_bf16(i16(x*A + B))
SCH_A = SCALE * (2.0**7) / math.log(2.0)
SCH_B = 127.0 * 2.0**7 - 7.4
I16 = mybir.dt.int16

LAST_RESULTS = None
_CACHE = {}


def _build_consts():
    # c[p, :] = causal keep-mask for a diagonal block: 1 where col >= p.
    rows = np.arange(P)[:, None]
    cols = np.arange(P)[None, :]
    return np.where(cols >= rows, np.float32(1.0), np.float32(0.0)).astype(BF16NP)


_CONSTS = _build_consts()


def build_module():
    nc = bacc.Bacc(None, target_bir_lowering=False)

    q_d = nc.dram_tensor("q", [NQH, D, SQ], BF16, kind="ExternalInput")
    k_d = nc.dram_tensor("k", [NKVH, D, SK], BF16, kind="ExternalInput")
    v_d = nc.dram_tensor("v", [NKVH, P, NKB, D + 1], BF16, kind="ExternalInput")
    c_d = nc.dram_tensor("c", [P, P], BF16, kind="ExternalInput")
    o_d = nc.dram_tensor("o", [NQH, NSB, P, 4 * (D + 1)], F32, kind="ExternalOutput")

    # per-head DVE (Schraudolph) pair quota per superblock; diagonal pairs
    # (the last two of each sb) always run on ACT.
    DVE_QUOTA = [0, 2, 2, 3]  # of the 2*sb off-diagonal pairs

    with tile.TileContext(nc) as tc:
        with (
            tc.tile_pool(name="const", bufs=1) as constp,
            tc.tile_pool(name="kt", bufs=2) as ktp,
            tc.tile_pool(name="qt", bufs=2) as qtp,
            tc.tile_pool(name="vaug", bufs=2) as vap,
            tc.tile_pool(name="pt", bufs=8) as ptp,
            tc.tile_pool(name="ti", bufs=6) as tip,
            tc.tile_pool(name="outs", bufs=3) as outp,
            tc.tile_pool(name="pst", bufs=4, space="PSUM") as pstp,
            tc.tile_pool(name="ppv", bufs=2, space="PSUM") as ppvp,
        ):
            # Host-provided causal keep-mask for diagonal 128x128 blocks.
            dmask = constp.tile([P, P], BF16, tag="dmask")
            nc.sync.dma_start(dmask[:], c_d[:])

            dve_diag_ctr = [0]

            def head_compute(h, qt, kt_g, vaug_g):
                for sb in range(NSB):
                    npairs = 2 * sb + 2
                    # two half-superblock PV accumulators, one bank per qb
                    pvs = [
                        ppvp.tile([P, 2, SBW], F32, tag="ppv", name=f"pv_{h}_{sb}_{hh}")
                        for hh in range(2)
                    ]
                    ot = outp.tile([P, 4, D + 1], F32, tag="outs")
                    for pair in range(npairs):
                        is_diag = pair >= 2 * sb
                        pth = [None, None]
                        for half in (0, 1):
                            kb = 2 * pair + half
                            st = pstp.tile([P, SBW], F32, tag="pst")
                            nc.tensor.matmul(
                                st[:],
                                kt_g[:, kb * P : (kb + 1) * P],
                                qt[:, sb * SBW : (sb + 1) * SBW],
                                start=True,
                                stop=True,
                            )
                            on_act = is_diag or half == 0
                            if is_diag and half == 1:
                                # shift a fraction of diagonal halves to DVE
                                # to balance ACT vs DVE busy time
                                if dve_diag_ctr[0] % 8 in (0, 3, 6):
                                    on_act = False
                                dve_diag_ctr[0] += 1
                            if on_act:
                                # ACT exp for this k-block half
                                ptt = ptp.tile([P, SBW], BF16, tag="pt")
                                nc.scalar.activation(
                                    ptt[:],
                                    st[:],
                                    mybir.ActivationFunctionType.Exp,
                                    scale=SCALE,
                                )
                                pth[half] = ptt[:]
                            else:
                                # DVE bf16-width Schraudolph exp: the int16
                                # result IS the bf16 bit pattern.
                                ti = tip.tile([P, SBW], I16, tag="ti")
                                nc.vector.tensor_scalar(
                                    out=ti[:],
                                    in0=st[:],
                                    scalar1=float(SCH_A),
                                    scalar2=float(SCH_B),
                                    op0=mybir.AluOpType.mult,
                                    op1=mybir.AluOpType.add,
                                )
                                pth[half] = ti[:].bitcast(BF16)
                            if is_diag:
                                # mask the diagonal 128x128 block kb == qb
                                j = kb - 4 * sb
                                nc.vector.tensor_tensor(
                                    out=pth[half][:, j * P : (j + 1) * P],
                                    in0=pth[half][:, j * P : (j + 1) * P],
                                    in1=dmask[:],
                                    op=mybir.AluOpType.mult,
                                )
                        for half in (0, 1):
                            kb = 2 * pair + half
                            for j in range(4):
                                qb = 4 * sb + j
                                if kb > qb:
                                    continue
                                nc.tensor.matmul(
                                    pvs[j // 2][:, j % 2, 0 : D + 1],
                                    pth[half][:, j * P : (j + 1) * P],
                                    vaug_g[:, kb, :],
                                    start=(kb == 0),
                                    stop=(kb == qb),
                                )
                        if pair == 2 * sb:
                            # half-superblock 0 (qb = 4sb, 4sb+1) is complete
                            nc.vector.tensor_copy(
                                ot[:, 0:2, :], pvs[0][:, :, 0 : D + 1]
                            )
                    nc.vector.tensor_copy(ot[:, 2:4, :], pvs[1][:, :, 0 : D + 1])
                    nc.sync.dma_start(
                        o_d[h, sb], ot[:].rearrange("p a b -> p (a b)")
                    )

            for g in range(NKVH):
                kt_g = ktp.tile([P, SK], BF16, tag="kt")
                vaug_g = vap.tile([P, NKB, D + 1], BF16, tag="vaug")
                # load order tuned for ramp: first superblock's operands first
                nc.sync.dma_start(kt_g[:, 0:SBW], k_d[g, :, 0:SBW])
                for hl in range(2):
                    h = 2 * g + hl
                    qt = qtp.tile([P, SQ], BF16, tag="qt")
                    nc.sync.dma_start(qt[:, 0:SBW], q_d[h, :, 0:SBW])
                    if hl == 0:
                        nc.sync.dma_start(vaug_g[:, 0:4], v_d[g, :, 0:4])
                        nc.sync.dma_start(kt_g[:, SBW:SK], k_d[g, :, SBW:SK])
                        nc.sync.dma_start(vaug_g[:, 4:NKB], v_d[g, :, 4:NKB])
                    nc.sync.dma_start(qt[:, SBW:SQ], q_d[h, :, SBW:SQ])
                    head_compute(h, qt[:], kt_g[:], vaug_g[:])

    nc.finalize()
    return nc


def _get_module():
    if "nc" not in _CACHE:
        _CACHE["nc"] = build_module()
    return _CACHE["nc"]


def kernel(q, kv):
    global LAST_RESULTS
    q = np.asarray(q, dtype=np.float32)
    kv = np.asarray(kv, dtype=np.float32)

    nc = _get_module()
    in_maps = []
    for c in range(NCORES):
        b, j = divmod(c, 4)
        # qT: [NQH, D, SQ]
        q_s = np.ascontiguousarray(
            np.transpose(q[b][:, 4 * j : 4 * j + 4, :], (1, 2, 0))
        ).astype(BF16NP)
        # kT: [NKVH, D, SK]
        k_s = np.ascontiguousarray(
            np.transpose(kv[b][:, 0, 2 * j : 2 * j + 2, :], (1, 2, 0))
        ).astype(BF16NP)
        # va: [NKVH, P, NKB, D+1] with ones at d=D
        v_full = kv[b][:, 1, 2 * j : 2 * j + 2, :]  # [SK, 2, D]
        va = np.ones((NKVH, P, NKB, D + 1), dtype=BF16NP)
        va[..., :D] = (
            np.transpose(v_full.reshape(NKB, P, NKVH, D), (2, 1, 0, 3))
        ).astype(BF16NP)
        in_maps.append({"q": q_s, "k": k_s, "v": va, "c": _CONSTS})

    trace = bool(int(os.environ.get("KERNEL_TRACE", "0")))
    kwargs = {}
    tdir = os.environ.get("KERNEL_TRACE_DIR")
    if tdir:
        kwargs["tmpdir"] = tdir
    res = run_bass_kernel_spmd(
        nc, in_maps, core_ids=list(range(NCORES)), trace=trace, **kwargs
    )
    LAST_RESULTS = res

    out = np.empty((B, SQ, H, D), np.float32)
    for c in range(NCORES):
        b, j = divmod(c, 4)
        o = res.results[c]["o"].reshape(NQH, NSB, P, 4, D + 1)
        o = np.transpose(o, (0, 1, 3, 2, 4)).reshape(NQH, SQ, D + 1)
        norm = o[..., :D] / o[..., D : D + 1]
        out[b, :, 4 * j : 4 * j + 4, :] = np.transpose(norm, (1, 0, 2))
    return out
